# revision 21
# baseline (speedup 1.0000x reference)
"""TRN2 Bass kernel for nn_Attention_369367187796 (Gram-matrix restructure).

Reference computation (B=4, DX=1024, N=4096, DQ=DK=DV=1024, fp32):
    Q = Wq @ x[b]; K = Wk @ x[b]; V = Wv @ x[b]
    scores = Q @ K.T   (contract n)
    p = softmax(scores / sqrt(DQ), axis=q)               <- softmax over q!
    out[q,n] = sum_k p[q,k] V[k,n]

Key algebra: Q, K, V are never needed explicitly.
    scores = (Wk x)(Wq x)^T = Wk G Wq^T      with G = x x^T  [dx, dx]
    out    = (P Wv) x                        with P = softmax(scores)
This replaces the five N-sized matmuls (5 * 2*1024*1024*4096 FLOP per
batch) with one N-sized Gram matmul + one N-sized output matmul + three
tiny 1024^3 matmuls: ~1.8x less tensor-engine work, and no DRAM spills
at all (G, T1T, p, PWT all fit in SBUF).

Sharding: 8 cores = 4 batches x 2 k-halves. Each core computes G (shared
work, duplicated within the pair), its k-half of scoresT[k, q] (softmax
over q is the free axis -> fully local), PWT[d,q] = sum_{k in half}
Wv[k,d] p[k,q], and the partial out = PWT^T x. Host sums the two partials.

Precision (numpy-simulated rel err 8.5e-4 vs fp64; tolerance 2e-2):
  - all matmuls single-pass f32r (fp32 @ 11 mantissa bits, full PE rate)
  - W mean removal: host ships Wq/Wk minus 0.5. scores expands into
    Wk' G Wq'^T + 0.5(1G)Wq'^T + [terms constant across q that cancel in
    softmax]. The second term is a row r[q] = 0.5 c Wq'^T (c = colsum G),
    computed with hi/lo f32r splits of c and r (their magnitudes are ~25x
    the score std, so single f32r would inject visible logit noise), and
    broadcast into the score psums via a C=1 ones matmul.
  - G symmetric: only upper 12 of 16 [128,512] blocks computed; the lower
    4 are PE-transposed mirrors (exactly preserves symmetry).

Layouts (per core):
    G[d',d]:      lhsT = xT tile [n, d'], rhs = xT tile [n, d]
    T1T[d,k]:     lhsT = G [d'-part, d], rhs = WkT' [d', k]   (contract d')
    scoresT[k,q]: lhsT = T1T [d, k], rhs = WqT' [d, q]        (contract d)
    PWT[d,q]:     lhsT = Wv rows [k, d], rhs = pT [k, q]      (contract k)
    out[q,n]:     lhsT = PWT [d, q], rhs = x [d, n]           (contract d)
xT is streamed once (host ships x[b].T); its low-d half stays SBUF
resident for the second Gram round. The walrus verifier requires f32r
matmul operands to come from a rounding compute op, so every DMA-landed
tile gets a cheap in-place f32r tensor_copy.
"""

import math

import numpy as np

B_FULL, DX_FULL, N_FULL = 4, 1024, 4096
DQ_FULL = DK_FULL = 1024
N_CORES = 8


def _build_core_kernel(DX, N, DQ, DKH, bench=False, bench_reps=0):
    import concourse.bass as bass
    import concourse.mybir as mybir
    import concourse.tile as tile
    from concourse import bacc
    from concourse.masks import make_identity

    f32 = mybir.dt.float32
    f32r = mybir.dt.float32r

    P = 128
    DT = DX // P            # d-tiles (8)
    NT = N // P             # n-tiles (32)
    KT = DKH // P           # k-tiles for this half (4)
    QT = DQ // P            # q-tiles (8)
    DH = DX // 2            # 512: G column split
    DHT = DT // 2           # 4
    scale = 1.0 / math.sqrt(DQ)

    assert DX % P == 0 and N % P == 0 and DQ % P == 0 and DKH % P == 0
    assert DX == DQ  # layout assumptions below

    nc = bacc.Bacc(None, target_bir_lowering=False, debug=False)

    kind_big = "Internal" if bench else "ExternalInput"
    kind_out = "Internal" if bench else "ExternalOutput"
    # f32r DRAM: same fp32 bits; SBUF tiles re-round after DMA
    xb = nc.dram_tensor("xb", [DX, N], f32r, kind=kind_big)
    xt = nc.dram_tensor("xt", [N, DX], f32r, kind=kind_big)
    wqt = nc.dram_tensor("wqt", [DX, DQ], f32r, kind=kind_big)    # Wq.T - .5
    wkt = nc.dram_tensor("wkt", [DX, DKH], f32r, kind=kind_big)   # Wk.T - .5
    wv = nc.dram_tensor("wv", [DKH, DX], f32r, kind=kind_big)     # Wv rows
    # tiny input consumed into one output element (value 0 at rest): lets a
    # benchmark chain data dependencies between repeated NEFF executions
    seed = nc.dram_tensor("seed", [1, 1], f32, kind="ExternalInput")
    out = nc.dram_tensor("out", [DQ, N], f32, kind=kind_out)
    sink = (nc.dram_tensor("sink", [1, 1], f32, kind="ExternalOutput")
            if bench else None)

    xv = xb.ap().rearrange("(dt p) n -> p dt n", p=P)
    xtv = xt.ap().rearrange("(nt p) d -> p nt d", p=P)
    wqv = wqt.ap().rearrange("(dt p) q -> p dt q", p=P)
    wkv = wkt.ap().rearrange("(dt p) k -> p dt k", p=P)
    wvv = wv.ap().rearrange("(kt p) d -> p kt d", p=P)
    outv = out.ap().rearrange("(qt p) n -> p qt n", p=P)

    with tile.TileContext(nc) as tc:
        with (
            tc.tile_pool(name="ps", bufs=8, space="PSUM") as ps,
            tc.tile_pool(name="pconst", bufs=1) as pconst,
        ):
            # constants (loop-invariant): identity for PE transpose, ones
            ident = pconst.tile([P, P], f32r, tag="ident")
            ident_st = pconst.tile([P, P], f32, tag="ident_st")
            make_identity(nc, ident_st[:])
            nc.vector.tensor_copy(ident[:], ident_st[:])
            ones_c = pconst.tile([P, 2], f32r, tag="ones_c")
            ones_row = pconst.tile([1, P], f32r, tag="ones_row")
            nc.gpsimd.memset(ident_st[:], 1.0)
            nc.vector.tensor_copy(ones_row[:], ident_st[0:1, :])
            nc.vector.tensor_copy(ones_c[:], ident_st[:, 0:2])

            rep_cm = tc.For_i(0, bench_reps, 1) if bench_reps else None
            if rep_cm is not None:
                rep_cm.__enter__()

            with (
                tc.tile_pool(name="pwqk", bufs=1) as pwqk,
                tc.tile_pool(name="pt", bufs=1) as pt,
            ):
                wq_r = pwqk.tile([P, DT, DQ], f32r, tag="wq")    # 32KB/p
                wk_r = pwqk.tile([P, DT, DKH], f32r, tag="wk")   # 16KB/p
                c_h = pwqk.tile([P, DT], f32r, tag="ch")
                c_l = pwqk.tile([P, DT], f32r, tag="cl")
                c05 = pwqk.tile([P, DT], f32, tag="c05")
                r_h = pwqk.tile([1, DQ], f32r, tag="rh")
                r_l = pwqk.tile([1, DQ], f32r, tag="rl")
                t1t = pt.tile([P, DT, DKH], f32r, tag="t1t")     # 16KB/p

                # ------------- Phase A: G = xT^T @ xT (+ c, r) -------------
                with (
                    tc.tile_pool(name="pg", bufs=1) as pg,
                    tc.tile_pool(name="pxh", bufs=1) as pxh,
                    tc.tile_pool(name="pxa", bufs=4) as pxa,
                ):
                    g_sb = pg.tile([P, DT, DX], f32r, tag="g")       # 32KB/p
                    xtr_half = pxh.tile([P, NT, DH], f32r, tag="xh")  # 64KB/p
                    GRP = 2
                    NG = NT // GRP
                    # round 1: G[:, DH:DX], all 8 d'-tiles (8 psum banks)
                    gps = [ps.tile([P, DH], f32, tag="ps", name=f"gps{d}")
                           for d in range(DT)]
                    for g in range(NG):
                        gsl = bass.ds(g * GRP, GRP)
                        xg = pxa.tile([P, GRP, DX], f32r, tag="xg",
                                      name=f"xg{g}")
                        nc.sync.dma_start(xg[:], xtv[:, gsl])
                        # re-round halves on different engines (concurrent)
                        nc.vector.tensor_copy(xtr_half[:, gsl], xg[:, :, 0:DH])
                        nc.gpsimd.tensor_copy(xg[:, :, DH:DX], xg[:, :, DH:DX])
                        for t in range(GRP):
                            nt = g * GRP + t
                            for dp in range(DT):
                                if dp < DHT:
                                    lhs = xtr_half[:, nt, bass.ds(dp * P, P)]
                                else:
                                    lhs = xg[:, t, bass.ds(dp * P, P)]
                                nc.tensor.matmul(
                                    gps[dp][:], lhs, xg[:, t, DH:DX],
                                    start=(nt == 0), stop=(nt == NT - 1))
                    for dp in range(DT):
                        nc.vector.tensor_copy(g_sb[:, dp, DH:DX], gps[dp][:])

                    # Wq/Wk loads: on the sync queue AFTER the xt stream, so
                    # they overlap the DMA-free Gram round 2 below
                    for dt in range(DT):
                        nc.sync.dma_start(wq_r[:, dt], wqv[:, dt])
                        nc.vector.tensor_copy(wq_r[:, dt], wq_r[:, dt])
                        if dt % 2 == 0:
                            d2 = bass.ds(dt, 2)
                            nc.sync.dma_start(wk_r[:, d2], wkv[:, d2])
                            nc.vector.tensor_copy(wk_r[:, d2], wk_r[:, d2])

                    # round 2: G[0:4 tiles, 0:DH] from resident half, no DMA
                    g2ps = [ps.tile([P, DH], f32, tag="ps", name=f"g2ps{d}")
                            for d in range(DHT)]
                    for nt in range(NT):
                        for dp in range(DHT):
                            nc.tensor.matmul(
                                g2ps[dp][:],
                                xtr_half[:, nt, bass.ds(dp * P, P)],
                                xtr_half[:, nt], start=(nt == 0),
                                stop=(nt == NT - 1))
                    for dp in range(DHT):
                        nc.vector.tensor_copy(g_sb[:, dp, 0:DH], g2ps[dp][:])
                    # mirror lower-left: g_sb[4+i, 128j:] = T(g_sb[j, DH+128i:])
                    for i in range(DHT):
                        for j in range(DHT):
                            tp = ps.tile([P, P], f32r, tag="ps",
                                         name=f"tp{i}_{j}")
                            nc.tensor.transpose(
                                tp[:], g_sb[:, j, bass.ds(DH + i * P, P)],
                                ident[:])
                            nc.vector.tensor_copy(
                                g_sb[:, DHT + i, bass.ds(j * P, P)], tp[:])

                    # c[d] = 0.5 * colsum G (exact mean-restore), hi/lo split
                    # (moving free size 2: fp32r matmuls reject F=1)
                    for dt in range(DT):
                        cps = ps.tile([P, 2], f32, tag="ps", name=f"cps{dt}")
                        for dp in range(DT):
                            nc.tensor.matmul(
                                cps[:], g_sb[:, dp, bass.ds(dt * P, P)],
                                ones_c[:], start=(dp == 0),
                                stop=(dp == DT - 1))
                        nc.vector.tensor_scalar_mul(c05[:, dt:dt + 1],
                                                    cps[:, 0:1], 0.5)
                    nc.vector.tensor_copy(c_h[:], c05[:])
                    nc.vector.tensor_sub(c_l[:], c05[:], c_h[:])
                    # r[q] = (c_h + c_l) @ Wq', hi/lo split
                    for qc in range(DQ // DH):
                        qsl = bass.ds(qc * DH, DH)
                        rps = ps.tile([1, DH], f32, tag="ps", name=f"rps{qc}")
                        for dt in range(DT):
                            nc.tensor.matmul(rps[:], c_h[:, dt:dt + 1],
                                             wq_r[:, dt, qsl],
                                             start=(dt == 0), stop=False)
                            nc.tensor.matmul(rps[:], c_l[:, dt:dt + 1],
                                             wq_r[:, dt, qsl],
                                             start=False, stop=(dt == DT - 1))
                        nc.vector.tensor_copy(r_h[:, qsl], rps[:])
                        nc.vector.tensor_sub(r_l[:, qsl], rps[:], r_h[:, qsl])

                    # ------------ Phase B: T1T[d,k] = G^T Wk' ------------
                    for dt in range(DT):
                        t1ps = ps.tile([P, DKH], f32, tag="ps",
                                       name=f"t1ps{dt}")
                        for dp in range(DT):
                            nc.tensor.matmul(
                                t1ps[:], g_sb[:, dp, bass.ds(dt * P, P)],
                                wk_r[:, dp], start=(dp == 0),
                                stop=(dp == DT - 1))
                        nc.vector.tensor_copy(t1t[:, dt], t1ps[:])
                # pg/pxh/pxa closed: g_sb and xT buffers freed

                with (
                    tc.tile_pool(name="ppwt", bufs=1) as ppwt,
                    tc.tile_pool(name="pwv", bufs=1) as pwv,
                ):
                    pwt = ppwt.tile([P, DT, DQ], f32r, tag="pwt")  # 32KB/p
                    wv_r = pwv.tile([P, KT, DX], f32r, tag="wv")   # 16KB/p
                    # Wv load overlaps phase C compute
                    for kt in range(KT):
                        nc.sync.dma_start(wv_r[:, kt], wvv[:, kt])
                        nc.vector.tensor_copy(wv_r[:, kt], wv_r[:, kt])

                    with (
                        tc.tile_pool(name="psc", bufs=1) as psc,
                        tc.tile_pool(name="psmx", bufs=2) as psmx,
                        tc.tile_pool(name="pstat", bufs=2) as pstat,
                    ):
                        # -------- Phase C: scoresT + softmax over q --------
                        scores_sb = psc.tile([P, KT, DQ], f32, tag="sc")
                        p_r = psc.tile([P, KT, DQ], f32r, tag="pr")
                        for kt in range(KT):
                            for qc in range(DQ // DH):
                                qsl = bass.ds(qc * DH, DH)
                                sp = ps.tile([P, DH], f32, tag="ps",
                                             name=f"sps{kt}_{qc}")
                                for dt in range(DT):
                                    nc.tensor.matmul(
                                        sp[:], t1t[:, dt, bass.ds(kt * P, P)],
                                        wq_r[:, dt, qsl],
                                        start=(dt == 0), stop=False)
                                nc.tensor.matmul(sp[:], ones_row[:],
                                                 r_h[:, qsl],
                                                 start=False, stop=False)
                                nc.tensor.matmul(sp[:], ones_row[:],
                                                 r_l[:, qsl],
                                                 start=False, stop=True)
                                nc.vector.tensor_copy(scores_sb[:, kt, qsl],
                                                      sp[:])
                            m = pstat.tile([P, 1], f32, tag="m")
                            negm = pstat.tile([P, 1], f32, tag="negm")
                            den = pstat.tile([P, 1], f32, tag="den")
                            rden = pstat.tile([P, 1], f32, tag="rden")
                            nc.vector.reduce_max(m[:], scores_sb[:, kt],
                                                 axis=mybir.AxisListType.X)
                            nc.vector.tensor_scalar_mul(negm[:], m[:], -scale)
                            e = psmx.tile([P, DQ], f32, tag="e")
                            nc.scalar.activation(
                                e[:], scores_sb[:, kt],
                                mybir.ActivationFunctionType.Exp,
                                bias=negm[:], scale=scale, accum_out=den[:])
                            nc.vector.reciprocal(rden[:], den[:])
                            nc.gpsimd.tensor_scalar_mul(p_r[:, kt], e[:],
                                                        rden[:])

                        # -------- Phase D: PWT[d,q] = Wv^T pT --------
                        for dt in range(DT):
                            for qc in range(DQ // DH):
                                qsl = bass.ds(qc * DH, DH)
                                pp = ps.tile([P, DH], f32, tag="ps",
                                             name=f"pps{dt}_{qc}")
                                for kt in range(KT):
                                    nc.tensor.matmul(
                                        pp[:],
                                        wv_r[:, kt, bass.ds(dt * P, P)],
                                        p_r[:, kt, qsl],
                                        start=(kt == 0), stop=(kt == KT - 1))
                                nc.vector.tensor_copy(pwt[:, dt, qsl], pp[:])
                    # psc/psmx/pstat closed

                    # ------------ Phase E: out = PWT^T @ x ------------
                    with (
                        tc.tile_pool(name="pxe", bufs=3) as pxe,
                        tc.tile_pool(name="pout", bufs=4) as pout,
                        tc.tile_pool(name="pseed", bufs=1) as pseed,
                    ):
                        seed_sb = pseed.tile([1, 1], f32, tag="seed")
                        nc.sync.dma_start(seed_sb[:], seed.ap())
                        for c in range(N // DH):
                            ncol = bass.ds(c * DH, DH)
                            xc = pxe.tile([P, DT, DH], f32r, tag="xe",
                                          name=f"xe{c}")
                            nc.sync.dma_start(xc[:], xv[:, :, ncol])
                            # re-round on the Pool engine: keeps DVE free
                            nc.gpsimd.tensor_copy(xc[:], xc[:])
                            for qg in range(QT // 4):
                                osb = pout.tile([P, 4, DH], f32, tag="osb")
                                for qi in range(4):
                                    qt = qg * 4 + qi
                                    op = ps.tile([P, DH], f32, tag="ps",
                                                 name=f"ops{c}_{qt}")
                                    for dt in range(DT):
                                        nc.tensor.matmul(
                                            op[:],
                                            pwt[:, dt, bass.ds(qt * P, P)],
                                            xc[:, dt],
                                            start=(dt == 0),
                                            stop=(dt == DT - 1))
                                    nc.vector.tensor_copy(osb[:, qi], op[:])
                                    if c == 0 and qt == 0:
                                        nc.vector.tensor_scalar_add(
                                            osb[0:1, 0, 0:1], op[0:1, 0:1],
                                            seed_sb[:])
                                        if sink is not None:
                                            nc.sync.dma_start(
                                                sink.ap(), osb[0:1, 0, 0:1])
                                nc.gpsimd.dma_start(
                                    outv[:, qg * 4:(qg + 1) * 4, ncol],
                                    osb[:])

            if rep_cm is not None:
                rep_cm.__exit__(None, None, None)

    nc.compile()
    return nc


_CACHE = {}


def _get_nc(DX, N, DQ, DKH):
    key = (DX, N, DQ, DKH)
    if key not in _CACHE:
        _CACHE[key] = _build_core_kernel(DX, N, DQ, DKH)
    return _CACHE[key]


def _run(x, Wq, Wk, Wv, **spmd_kwargs):
    """Run the SPMD kernel; returns (out, BassKernelResults)."""
    from concourse.bass_utils import run_bass_kernel_spmd

    B, DX, N = x.shape
    DQ = Wq.shape[0]
    DK = Wk.shape[0]
    assert (B, DX, N, DQ, DK) == (B_FULL, DX_FULL, N_FULL, DQ_FULL, DK_FULL)
    DKH = DK // 2

    nc = _get_nc(DX, N, DQ, DKH)

    # Wq/Wk shipped mean-removed (entries - 0.5); the kernel restores the
    # exact mean term via the colsum-of-G row (see builder docstring)
    WqT = np.ascontiguousarray(Wq.T, dtype=np.float32) - np.float32(0.5)
    WkT = np.ascontiguousarray(Wk.T, dtype=np.float32) - np.float32(0.5)

    in_maps = []
    for c in range(N_CORES):
        b, h = divmod(c, 2)
        hsl = slice(h * DKH, (h + 1) * DKH)
        in_maps.append({
            "xb": np.ascontiguousarray(x[b], dtype=np.float32),
            "xt": np.ascontiguousarray(x[b].T, dtype=np.float32),
            "wqt": WqT,
            "wkt": np.ascontiguousarray(WkT[:, hsl]),
            "wv": np.ascontiguousarray(Wv[hsl, :], dtype=np.float32),
            "seed": np.zeros((1, 1), np.float32),
        })

    res = run_bass_kernel_spmd(nc, in_maps, core_ids=list(range(N_CORES)),
                               **spmd_kwargs)
    out = np.empty((B, DQ, N), np.float32)
    for b in range(B):
        out[b] = res.results[2 * b]["out"] + res.results[2 * b + 1]["out"]
    return out, res


def kernel(x, Wq, Wk, Wv):
    return _run(x, Wq, Wk, Wv)[0]


# revision 22
# speedup vs baseline: 1.4272x; 1.4272x over previous
"""TRN2 Bass kernel for nn_Attention_369367187796 (Gram-matrix restructure).

Reference computation (B=4, DX=1024, N=4096, DQ=DK=DV=1024, fp32):
    Q = Wq @ x[b]; K = Wk @ x[b]; V = Wv @ x[b]
    scores = Q @ K.T   (contract n)
    p = softmax(scores / sqrt(DQ), axis=q)               <- softmax over q!
    out[q,n] = sum_k p[q,k] V[k,n]

Key algebra: Q, K, V are never needed explicitly.
    scores = (Wk x)(Wq x)^T = Wk G Wq^T      with G = x x^T  [dx, dx]
    out    = (P Wv) x                        with P = softmax(scores)
This replaces the five N-sized matmuls (5 * 2*1024*1024*4096 FLOP per
batch) with one N-sized Gram matmul + one N-sized output matmul + three
tiny 1024^3 matmuls: ~1.8x less tensor-engine work, and no DRAM spills
at all (G, T1T, p, PWT all fit in SBUF).

Sharding: 8 cores = 4 batches x 2 k-halves. Each core computes G (shared
work, duplicated within the pair), its k-half of scoresT[k, q] (softmax
over q is the free axis -> fully local), PWT[d,q] = sum_{k in half}
Wv[k,d] p[k,q], and the partial out = PWT^T x. Host sums the two partials.

Precision (numpy-simulated rel err 8.5e-4 vs fp64; tolerance 2e-2):
  - all matmuls single-pass f32r (fp32 @ 11 mantissa bits, full PE rate)
  - W mean removal: host ships Wq/Wk minus 0.5. scores expands into
    Wk' G Wq'^T + 0.5(1G)Wq'^T + [terms constant across q that cancel in
    softmax]. The second term is a row r[q] = 0.5 c Wq'^T (c = colsum G),
    computed with hi/lo f32r splits of c and r (their magnitudes are ~25x
    the score std, so single f32r would inject visible logit noise), and
    broadcast into the score psums via a C=1 ones matmul.
  - G symmetric: only upper 12 of 16 [128,512] blocks computed; the lower
    4 are PE-transposed mirrors (exactly preserves symmetry).

Layouts (per core):
    G[d',d]:      lhsT = xT tile [n, d'], rhs = xT tile [n, d]
    T1T[d,k]:     lhsT = G [d'-part, d], rhs = WkT' [d', k]   (contract d')
    scoresT[k,q]: lhsT = T1T [d, k], rhs = WqT' [d, q]        (contract d)
    PWT[d,q]:     lhsT = Wv rows [k, d], rhs = pT [k, q]      (contract k)
    out[q,n]:     lhsT = PWT [d, q], rhs = x [d, n]           (contract d)
xT is streamed once (host ships x[b].T); its low-d half stays SBUF
resident for the second Gram round. The walrus verifier requires f32r
matmul operands to come from a rounding compute op, so every DMA-landed
tile gets a cheap in-place f32r tensor_copy.
"""

import math

import numpy as np

B_FULL, DX_FULL, N_FULL = 4, 1024, 4096
DQ_FULL = DK_FULL = 1024
N_CORES = 8


def _build_core_kernel(DX, N, DQ, DKH, bench=False, bench_reps=0):
    import concourse.bass as bass
    import concourse.mybir as mybir
    import concourse.tile as tile
    from concourse import bacc
    from concourse.masks import make_identity

    f32 = mybir.dt.float32
    f32r = mybir.dt.float32r

    P = 128
    DT = DX // P            # d-tiles (8)
    NT = N // P             # n-tiles (32)
    KT = DKH // P           # k-tiles for this half (4)
    QT = DQ // P            # q-tiles (8)
    DH = DX // 2            # 512: G column split
    DHT = DT // 2           # 4
    scale = 1.0 / math.sqrt(DQ)

    assert DX % P == 0 and N % P == 0 and DQ % P == 0 and DKH % P == 0
    assert DX == DQ  # layout assumptions below

    nc = bacc.Bacc(None, target_bir_lowering=False, debug=False)

    kind_big = "Internal" if bench else "ExternalInput"
    kind_out = "Internal" if bench else "ExternalOutput"
    # f32r DRAM: same fp32 bits; SBUF tiles re-round after DMA
    xb = nc.dram_tensor("xb", [DX, N], f32r, kind=kind_big)
    xt = nc.dram_tensor("xt", [N, DX], f32r, kind=kind_big)
    wqt = nc.dram_tensor("wqt", [DX, DQ], f32r, kind=kind_big)    # Wq.T - .5
    wkt = nc.dram_tensor("wkt", [DX, DKH], f32r, kind=kind_big)   # Wk.T - .5
    wv = nc.dram_tensor("wv", [DKH, DX], f32r, kind=kind_big)     # Wv rows
    # tiny input consumed into one output element (value 0 at rest): lets a
    # benchmark chain data dependencies between repeated NEFF executions
    seed = nc.dram_tensor("seed", [1, 1], f32, kind="ExternalInput")
    out = nc.dram_tensor("out", [DQ, N], f32, kind=kind_out)
    sink = (nc.dram_tensor("sink", [1, 1], f32, kind="ExternalOutput")
            if bench else None)

    xv = xb.ap().rearrange("(dt p) n -> p dt n", p=P)
    xtv = xt.ap().rearrange("(nt p) d -> p nt d", p=P)
    wqv = wqt.ap().rearrange("(dt p) q -> p dt q", p=P)
    wkv = wkt.ap().rearrange("(dt p) k -> p dt k", p=P)
    wvv = wv.ap().rearrange("(kt p) d -> p kt d", p=P)
    outv = out.ap().rearrange("(qt p) n -> p qt n", p=P)

    with tile.TileContext(nc) as tc:
        with (
            tc.tile_pool(name="ps", bufs=8, space="PSUM") as ps,
            tc.tile_pool(name="pconst", bufs=1) as pconst,
        ):
            # constants (loop-invariant): identity for PE transpose, ones
            ident = pconst.tile([P, P], f32r, tag="ident")
            ident_st = pconst.tile([P, P], f32, tag="ident_st")
            make_identity(nc, ident_st[:])
            nc.vector.tensor_copy(ident[:], ident_st[:])
            ones_c = pconst.tile([P, 2], f32r, tag="ones_c")
            ones_row = pconst.tile([1, P], f32r, tag="ones_row")
            nc.gpsimd.memset(ident_st[:], 1.0)
            nc.vector.tensor_copy(ones_row[:], ident_st[0:1, :])
            nc.vector.tensor_copy(ones_c[:], ident_st[:, 0:2])

            rep_cm = tc.For_i(0, bench_reps, 1) if bench_reps else None
            if rep_cm is not None:
                rep_cm.__enter__()

            with (
                tc.tile_pool(name="pwqk", bufs=1) as pwqk,
                tc.tile_pool(name="pt", bufs=1) as pt,
            ):
                wq_r = pwqk.tile([P, DT, DQ], f32r, tag="wq")    # 32KB/p
                wk_r = pwqk.tile([P, DT, DKH], f32r, tag="wk")   # 16KB/p
                c_h = pwqk.tile([P, DT], f32r, tag="ch")
                c_l = pwqk.tile([P, DT], f32r, tag="cl")
                c05 = pwqk.tile([P, DT], f32, tag="c05")
                r_h = pwqk.tile([1, DQ], f32r, tag="rh")
                r_l = pwqk.tile([1, DQ], f32r, tag="rl")
                t1t = pt.tile([P, DT, DKH], f32r, tag="t1t")     # 16KB/p

                # ------------- Phase A: G = xT^T @ xT (+ c, r) -------------
                with (
                    tc.tile_pool(name="pg", bufs=1) as pg,
                    tc.tile_pool(name="pxh", bufs=1) as pxh,
                    tc.tile_pool(name="pxa", bufs=4) as pxa,
                ):
                    g_sb = pg.tile([P, DT, DX], f32r, tag="g")       # 32KB/p
                    xtr_half = pxh.tile([P, NT, DH], f32r, tag="xh")  # 64KB/p
                    GRP = 2
                    NG = NT // GRP
                    # round 1: G[:, DH:DX], all 8 d'-tiles (8 psum banks)
                    gps = [ps.tile([P, DH], f32, tag="ps", name=f"gps{d}")
                           for d in range(DT)]
                    for g in range(NG):
                        gsl = bass.ds(g * GRP, GRP)
                        xg = pxa.tile([P, GRP, DX], f32r, tag="xg",
                                      name=f"xg{g}")
                        nc.sync.dma_start(xg[:], xtv[:, gsl])
                        # re-round: low-d half lands in the resident tile
                        nc.vector.tensor_copy(xtr_half[:, gsl], xg[:, :, 0:DH])
                        nc.vector.tensor_copy(xg[:, :, DH:DX], xg[:, :, DH:DX])
                        for t in range(GRP):
                            nt = g * GRP + t
                            for dp in range(DT):
                                if dp < DHT:
                                    lhs = xtr_half[:, nt, bass.ds(dp * P, P)]
                                else:
                                    lhs = xg[:, t, bass.ds(dp * P, P)]
                                nc.tensor.matmul(
                                    gps[dp][:], lhs, xg[:, t, DH:DX],
                                    start=(nt == 0), stop=(nt == NT - 1))
                    for dp in range(DT):
                        nc.vector.tensor_copy(g_sb[:, dp, DH:DX], gps[dp][:])

                    # Wq/Wk loads: on the sync queue AFTER the xt stream, so
                    # they overlap the DMA-free Gram round 2 below
                    for dt in range(DT):
                        nc.sync.dma_start(wq_r[:, dt], wqv[:, dt])
                        nc.vector.tensor_copy(wq_r[:, dt], wq_r[:, dt])
                        if dt % 2 == 0:
                            d2 = bass.ds(dt, 2)
                            nc.sync.dma_start(wk_r[:, d2], wkv[:, d2])
                            nc.vector.tensor_copy(wk_r[:, d2], wk_r[:, d2])

                    # round 2: G[0:4 tiles, 0:DH] from resident half, no DMA
                    g2ps = [ps.tile([P, DH], f32, tag="ps", name=f"g2ps{d}")
                            for d in range(DHT)]
                    for nt in range(NT):
                        for dp in range(DHT):
                            nc.tensor.matmul(
                                g2ps[dp][:],
                                xtr_half[:, nt, bass.ds(dp * P, P)],
                                xtr_half[:, nt], start=(nt == 0),
                                stop=(nt == NT - 1))
                    for dp in range(DHT):
                        nc.vector.tensor_copy(g_sb[:, dp, 0:DH], g2ps[dp][:])
                    # mirror lower-left: g_sb[4+i, 128j:] = T(g_sb[j, DH+128i:])
                    for i in range(DHT):
                        for j in range(DHT):
                            tp = ps.tile([P, P], f32r, tag="ps",
                                         name=f"tp{i}_{j}")
                            nc.tensor.transpose(
                                tp[:], g_sb[:, j, bass.ds(DH + i * P, P)],
                                ident[:])
                            nc.vector.tensor_copy(
                                g_sb[:, DHT + i, bass.ds(j * P, P)], tp[:])

                    # c[d] = 0.5 * colsum G (exact mean-restore), hi/lo split
                    # (moving free size 2: fp32r matmuls reject F=1)
                    for dt in range(DT):
                        cps = ps.tile([P, 2], f32, tag="ps", name=f"cps{dt}")
                        for dp in range(DT):
                            nc.tensor.matmul(
                                cps[:], g_sb[:, dp, bass.ds(dt * P, P)],
                                ones_c[:], start=(dp == 0),
                                stop=(dp == DT - 1))
                        nc.vector.tensor_scalar_mul(c05[:, dt:dt + 1],
                                                    cps[:, 0:1], 0.5)
                    nc.vector.tensor_copy(c_h[:], c05[:])
                    nc.vector.tensor_sub(c_l[:], c05[:], c_h[:])
                    # r[q] = (c_h + c_l) @ Wq', hi/lo split
                    for qc in range(DQ // DH):
                        qsl = bass.ds(qc * DH, DH)
                        rps = ps.tile([1, DH], f32, tag="ps", name=f"rps{qc}")
                        for dt in range(DT):
                            nc.tensor.matmul(rps[:], c_h[:, dt:dt + 1],
                                             wq_r[:, dt, qsl],
                                             start=(dt == 0), stop=False)
                            nc.tensor.matmul(rps[:], c_l[:, dt:dt + 1],
                                             wq_r[:, dt, qsl],
                                             start=False, stop=(dt == DT - 1))
                        nc.vector.tensor_copy(r_h[:, qsl], rps[:])
                        nc.vector.tensor_sub(r_l[:, qsl], rps[:], r_h[:, qsl])

                    # ------------ Phase B: T1T[d,k] = G^T Wk' ------------
                    for dt in range(DT):
                        t1ps = ps.tile([P, DKH], f32, tag="ps",
                                       name=f"t1ps{dt}")
                        for dp in range(DT):
                            nc.tensor.matmul(
                                t1ps[:], g_sb[:, dp, bass.ds(dt * P, P)],
                                wk_r[:, dp], start=(dp == 0),
                                stop=(dp == DT - 1))
                        nc.vector.tensor_copy(t1t[:, dt], t1ps[:])
                # pg/pxh/pxa closed: g_sb and xT buffers freed

                with (
                    tc.tile_pool(name="ppwt", bufs=1) as ppwt,
                    tc.tile_pool(name="pwv", bufs=1) as pwv,
                ):
                    pwt = ppwt.tile([P, DT, DQ], f32r, tag="pwt")  # 32KB/p
                    wv_r = pwv.tile([P, KT, DX], f32r, tag="wv")   # 16KB/p
                    # Wv load overlaps phase C compute
                    for kt in range(KT):
                        nc.sync.dma_start(wv_r[:, kt], wvv[:, kt])
                        nc.vector.tensor_copy(wv_r[:, kt], wv_r[:, kt])

                    with (
                        tc.tile_pool(name="psc", bufs=1) as psc,
                        tc.tile_pool(name="psmx", bufs=2) as psmx,
                        tc.tile_pool(name="pstat", bufs=2) as pstat,
                    ):
                        # -------- Phase C: scoresT + softmax over q --------
                        scores_sb = psc.tile([P, KT, DQ], f32, tag="sc")
                        p_r = psc.tile([P, KT, DQ], f32r, tag="pr")
                        for kt in range(KT):
                            for qc in range(DQ // DH):
                                qsl = bass.ds(qc * DH, DH)
                                sp = ps.tile([P, DH], f32, tag="ps",
                                             name=f"sps{kt}_{qc}")
                                for dt in range(DT):
                                    nc.tensor.matmul(
                                        sp[:], t1t[:, dt, bass.ds(kt * P, P)],
                                        wq_r[:, dt, qsl],
                                        start=(dt == 0), stop=False)
                                nc.tensor.matmul(sp[:], ones_row[:],
                                                 r_h[:, qsl],
                                                 start=False, stop=False)
                                nc.tensor.matmul(sp[:], ones_row[:],
                                                 r_l[:, qsl],
                                                 start=False, stop=True)
                                nc.vector.tensor_copy(scores_sb[:, kt, qsl],
                                                      sp[:])
                            m = pstat.tile([P, 1], f32, tag="m")
                            negm = pstat.tile([P, 1], f32, tag="negm")
                            den = pstat.tile([P, 1], f32, tag="den")
                            rden = pstat.tile([P, 1], f32, tag="rden")
                            nc.vector.reduce_max(m[:], scores_sb[:, kt],
                                                 axis=mybir.AxisListType.X)
                            nc.vector.tensor_scalar_mul(negm[:], m[:], -scale)
                            e = psmx.tile([P, DQ], f32, tag="e")
                            nc.scalar.activation(
                                e[:], scores_sb[:, kt],
                                mybir.ActivationFunctionType.Exp,
                                bias=negm[:], scale=scale, accum_out=den[:])
                            nc.vector.reciprocal(rden[:], den[:])
                            nc.vector.tensor_scalar_mul(p_r[:, kt], e[:],
                                                        rden[:])

                        # -------- Phase D: PWT[d,q] = Wv^T pT --------
                        for dt in range(DT):
                            for qc in range(DQ // DH):
                                qsl = bass.ds(qc * DH, DH)
                                pp = ps.tile([P, DH], f32, tag="ps",
                                             name=f"pps{dt}_{qc}")
                                for kt in range(KT):
                                    nc.tensor.matmul(
                                        pp[:],
                                        wv_r[:, kt, bass.ds(dt * P, P)],
                                        p_r[:, kt, qsl],
                                        start=(kt == 0), stop=(kt == KT - 1))
                                nc.vector.tensor_copy(pwt[:, dt, qsl], pp[:])
                    # psc/psmx/pstat closed

                    # ------------ Phase E: out = PWT^T @ x ------------
                    with (
                        tc.tile_pool(name="pxe", bufs=3) as pxe,
                        tc.tile_pool(name="pout", bufs=4) as pout,
                        tc.tile_pool(name="pseed", bufs=1) as pseed,
                    ):
                        seed_sb = pseed.tile([1, 1], f32, tag="seed")
                        nc.sync.dma_start(seed_sb[:], seed.ap())
                        for c in range(N // DH):
                            ncol = bass.ds(c * DH, DH)
                            xc = pxe.tile([P, DT, DH], f32r, tag="xe",
                                          name=f"xe{c}")
                            nc.sync.dma_start(xc[:], xv[:, :, ncol])
                            # re-round on the Pool engine: keeps DVE free
                            nc.gpsimd.tensor_copy(xc[:], xc[:])
                            for qg in range(QT // 4):
                                osb = pout.tile([P, 4, DH], f32, tag="osb")
                                for qi in range(4):
                                    qt = qg * 4 + qi
                                    op = ps.tile([P, DH], f32, tag="ps",
                                                 name=f"ops{c}_{qt}")
                                    for dt in range(DT):
                                        nc.tensor.matmul(
                                            op[:],
                                            pwt[:, dt, bass.ds(qt * P, P)],
                                            xc[:, dt],
                                            start=(dt == 0),
                                            stop=(dt == DT - 1))
                                    nc.vector.tensor_copy(osb[:, qi], op[:])
                                    if c == 0 and qt == 0:
                                        nc.vector.tensor_scalar_add(
                                            osb[0:1, 0, 0:1], op[0:1, 0:1],
                                            seed_sb[:])
                                        if sink is not None:
                                            nc.sync.dma_start(
                                                sink.ap(), osb[0:1, 0, 0:1])
                                nc.gpsimd.dma_start(
                                    outv[:, qg * 4:(qg + 1) * 4, ncol],
                                    osb[:])

            if rep_cm is not None:
                rep_cm.__exit__(None, None, None)

    nc.compile()
    return nc


_CACHE = {}


def _get_nc(DX, N, DQ, DKH):
    key = (DX, N, DQ, DKH)
    if key not in _CACHE:
        _CACHE[key] = _build_core_kernel(DX, N, DQ, DKH)
    return _CACHE[key]


def _run(x, Wq, Wk, Wv, **spmd_kwargs):
    """Run the SPMD kernel; returns (out, BassKernelResults)."""
    from concourse.bass_utils import run_bass_kernel_spmd

    B, DX, N = x.shape
    DQ = Wq.shape[0]
    DK = Wk.shape[0]
    assert (B, DX, N, DQ, DK) == (B_FULL, DX_FULL, N_FULL, DQ_FULL, DK_FULL)
    DKH = DK // 2

    nc = _get_nc(DX, N, DQ, DKH)

    # Wq/Wk shipped mean-removed (entries - 0.5); the kernel restores the
    # exact mean term via the colsum-of-G row (see builder docstring)
    WqT = np.ascontiguousarray(Wq.T, dtype=np.float32) - np.float32(0.5)
    WkT = np.ascontiguousarray(Wk.T, dtype=np.float32) - np.float32(0.5)

    in_maps = []
    for c in range(N_CORES):
        b, h = divmod(c, 2)
        hsl = slice(h * DKH, (h + 1) * DKH)
        in_maps.append({
            "xb": np.ascontiguousarray(x[b], dtype=np.float32),
            "xt": np.ascontiguousarray(x[b].T, dtype=np.float32),
            "wqt": WqT,
            "wkt": np.ascontiguousarray(WkT[:, hsl]),
            "wv": np.ascontiguousarray(Wv[hsl, :], dtype=np.float32),
            "seed": np.zeros((1, 1), np.float32),
        })

    res = run_bass_kernel_spmd(nc, in_maps, core_ids=list(range(N_CORES)),
                               **spmd_kwargs)
    out = np.empty((B, DQ, N), np.float32)
    for b in range(B):
        out[b] = res.results[2 * b]["out"] + res.results[2 * b + 1]["out"]
    return out, res


def kernel(x, Wq, Wk, Wv):
    return _run(x, Wq, Wk, Wv)[0]
